# revision 3
# baseline (speedup 1.0000x reference)
"""Trainium2 Bass kernel: mixture-of-Gaussians mean log-likelihood.

Computes mean_n logsumexp_k [ -0.5*quad(n,k) + c_k ] over N=2M points,
K=32 components, D=16 dims, data-parallel over 8 NeuronCores.

Math:  quad(n,k) = |M_k x_n - M_k mu_k|^2 with M_k = chol(A_k A_k^T)^{-1},
       c_k = -logdet_k - (D/2) log 2pi + w_k^2.
Device: per point z = G^T x~ (f16 matmuls -> [128, 512] f32 PSUM per
128-point tile), quad_k = sum_d z_{k,d}^2, s = sum_k W_k exp(-quad_k/2),
accumulate log s.  Host finishes: mean = (sum log s - pads)/N - SHIFT.

Engine split per gsize=4-tile group (z [128, 2048] f32 PSUM), cycling
PATTERN:
  'A' groups: ScalarE activation Square -> sq f16 SBUF, then an f16 fold
      chain 16->8->4->2->1 over d on DVE (2x_1P mode for all but last).
  'V' groups: custom DVE op SQ_CUMSUM (single-source scan of z^2, fp32
      state) straight from PSUM -> cumsum f32 SBUF; one strided
      tensor_sub then yields all 128 (tile,k) quads.  This drains PSUM
      without touching the Scalar engine (GPSIMD cannot read PSUM, and
      DVE tensor_tensor may read only one PSUM operand - the
      single-source custom op sidesteps both limits).
  Flush every TB tiles: exp on ScalarE, weight-mul on Pool (gpsimd),
  k-reduce on DVE; final batched Ln with accumulate.
"""

import re
from contextlib import ExitStack

import numpy as np

import concourse.bass as bass
import concourse.mybir as mybir
import concourse.tile as tile
from concourse import bacc, dve_ops
from concourse.bass_utils import run_bass_kernel_spmd
from concourse.dve_spec import AluOp, Spec, Src0, scan, sq

F32 = mybir.dt.float32
F16 = mybir.dt.float16

# Problem constants
N_TOTAL = 2_000_000
D = 16
K = 32
NCORES = 8
NC = N_TOTAL // NCORES
GROUPS = 3
CPART = D + 1
LOG_2PI = float(np.log(2.0 * np.pi))
SHIFT = 23.0

# Tiling (per core)
TPG = 656
NG = TPG * 128
NPC = GROUPS * NG
WCHUNK = 2048
TB = 24
PATTERN = "AAAAVAAAVA"  # square path per gsize-group
FOLD_PATTERN = "V"      # fold engine for successive 'A' groups

_MODULE_CACHE: dict = {}


def _make_custom_op(name, spec):
    op = dve_ops.DveOp(name, spec, subdim=False, uops_sha={})
    if not any(o.name == name for o in dve_ops.OPS):
        dve_ops.OPS.append(op)
        dve_ops.CUSTOM_DVE_SPECS[name] = spec
        dve_ops._SUB_OPCODE_FOR_NAME[name] = (
            dve_ops._CUSTOM_DVE_ROW_BASE + len(dve_ops.OPS) - 1)
    for ver in ("v3", "v4"):
        try:
            op.compile(ver)
        except ValueError as e:
            m = re.search(r'uops_sha\["%s"\]="([0-9a-f]+)"' % ver, str(e))
            if not m:
                raise
            op.uops_sha[ver] = m.group(1)
            op.compile(ver)
    return op


SQ_CUMSUM = _make_custom_op(
    "SQ_CUMSUM_ANT",
    Spec(body=scan(AluOp.ADD, sq(Src0)),
         reference=lambda in0, in1, c0, c1, c2:
         np.cumsum(in0.astype(np.float32) ** 2, axis=-1)))


def build_module(tpg: int = TPG, wchunk: int = WCHUNK, tb: int = TB,
                 reps: int = 1, gsize: int = 4, pattern: str = PATTERN,
                 fold_pattern: str = FOLD_PATTERN):
    """Device I/O per core: t [51, ng] f16, g [17, 512] f16,
    wrep [128, K] f32, out [128, 1] f32."""
    ng = tpg * 128
    assert ng % wchunk == 0
    nchunks = ng // wchunk
    tiles_pcg = wchunk // 128
    ntiles = GROUPS * tpg
    assert ntiles % tb == 0 and tb % gsize == 0 and tiles_pcg % gsize == 0

    nc = bacc.Bacc("TRN2", target_bir_lowering=False, debug=False)

    t_in = nc.dram_tensor("t", [GROUPS * CPART, ng], F16,
                          kind="ExternalInput").ap()
    g_in = nc.dram_tensor("g", [CPART, 2 * 256], F16,
                          kind="ExternalInput").ap()
    w_in = nc.dram_tensor("wrep", [128, K], F32, kind="ExternalInput").ap()
    out = nc.dram_tensor("out", [128, 1], F32, kind="ExternalOutput").ap()

    AX = mybir.AxisListType
    OP = mybir.AluOpType
    AF = mybir.ActivationFunctionType
    SEGS = gsize * K                     # cumsum segments per group

    with tile.TileContext(nc) as tc, ExitStack() as ctx:
        data_pool = ctx.enter_context(tc.tile_pool(name="data", bufs=2))
        zpool = ctx.enter_context(tc.tile_pool(name="z", bufs=2,
                                               space="PSUM"))
        sqpool = ctx.enter_context(tc.tile_pool(name="sq", bufs=4))
        foldpool = ctx.enter_context(tc.tile_pool(name="fold", bufs=4))
        cpool = ctx.enter_context(tc.tile_pool(name="const", bufs=1))

        gt = cpool.tile([CPART, 2 * 256], F16)
        nc.sync.dma_start(gt[:], g_in)
        wrept = cpool.tile([128, K], F32)
        nc.sync.dma_start(wrept[:], w_in)

        quad_buf = cpool.tile([128, 2, tb, K], F16)
        e_buf = cpool.tile([128, 2, tb, K], F32)
        ew_buf = cpool.tile([128, 2, tb, K], F32)
        s_buf = cpool.tile([128, ntiles], F32)
        ln_buf = cpool.tile([128, ntiles], F32)
        res = cpool.tile([128, 1], F32)
        cs_big = cpool.tile([128, 2, (SEGS + 1) * D], F32)

        wrep_bc = wrept[:].unsqueeze(1).broadcast_to([128, tb, K])
        csv = cs_big[:].rearrange("p v (s i) -> p v s i", i=D)
        # zero each slot's cumsum seed column once; the scan never writes it
        for v in range(2):
            nc.vector.memset(cs_big[:, v, D - 1:D], 0.0)

        def flush_batch(b):
            half = b % 2
            qv = quad_buf[:, half]          # [128, tb, K]
            ev = e_buf[:, half]
            ewv = ew_buf[:, half]
            nc.scalar.activation(ev, qv, AF.Exp, scale=-0.5)
            nc.gpsimd.tensor_mul(ewv, ev, wrep_bc)
            nc.vector.tensor_reduce(
                s_buf[:, b * tb:(b + 1) * tb], ewv, axis=AX.X, op=OP.add)

        g0r = gt[:, 0:256]
        g1r = gt[:, 256:512]

        def fold_chain(eng, sq, quad_t):
            # sq [128, gsize*512] f16, d innermost; fold d 16 -> 1
            red_in = sq[:].rearrange("p (t k i) -> p t k i", t=gsize, i=D)
            w = D
            while w > 1:
                w //= 2
                if w == 1:
                    ftile = quad_t.unsqueeze(3)
                else:
                    fold_t = foldpool.tile([128, gsize, K, w], F16,
                                           tag=f"fold{w}", name=f"fold{w}")
                    ftile = fold_t[:]
                eng.tensor_add(ftile, red_in[:, :, :, 0:w],
                               red_in[:, :, :, w:2 * w])
                red_in = ftile

        qflat = quad_buf[:].rearrange("p h t k -> p (h t k)")

        vidx = [0]

        def emit_main():
            t_idx = 0
            gidx = 0
            aidx = 0
            for ch in range(nchunks):
                csl = slice(ch * wchunk, (ch + 1) * wchunk)
                dts = []
                for gb in range(GROUPS):
                    dt_g = data_pool.tile([CPART, wchunk], F16,
                                          tag=f"dt{gb}")
                    nc.sync.dma_start(
                        dt_g[:], t_in[CPART * gb:CPART * (gb + 1), csl])
                    dts.append(dt_g)
                for g in range(GROUPS):
                    for j in range(0, tiles_pcg, gsize):
                        zt = zpool.tile([128, gsize * 512], F32)
                        for u in range(gsize):
                            lhsT = dts[g][:, (j + u) * 128:(j + u + 1) * 128]
                            nc.tensor.matmul(zt[:, u * 512:u * 512 + 256],
                                             lhsT, g0r, start=True, stop=True)
                            nc.tensor.matmul(
                                zt[:, u * 512 + 256:u * 512 + 512],
                                lhsT, g1r, start=True, stop=True)
                        b, tbi = divmod(t_idx, tb)
                        assert tbi + gsize <= tb
                        quad_t = quad_buf[:, b % 2, tbi:tbi + gsize]
                        eng = pattern[gidx % len(pattern)]
                        gidx += 1
                        if eng == "A":
                            z3 = zt[:].rearrange("p (h c) -> p h c",
                                                 h=2 * gsize)
                            sq = sqpool.tile([128, gsize * 512], F16)
                            sq3 = sq[:].rearrange("p (h c) -> p h c",
                                                  h=2 * gsize)
                            nc.scalar.activation(sq3, z3, AF.Square)
                            feng = fold_pattern[aidx % len(fold_pattern)]
                            aidx += 1
                            with nc.allow_low_precision("quad rounding ok"):
                                fold_chain(
                                    nc.vector if feng == "V" else nc.gpsimd,
                                    sq, quad_t)
                        else:   # 'V': DVE cumsum-of-squares + strided diff
                            v = vidx[0] % 2
                            vidx[0] += 1
                            nc.vector._custom_dve(
                                SQ_CUMSUM, out=cs_big[:, v, D:], in0=zt[:])
                            q2 = qflat[:, ((b % 2) * tb + tbi) * K:
                                       ((b % 2) * tb + tbi + gsize) * K]
                            with nc.allow_low_precision("quad rounding ok"):
                                nc.vector.tensor_sub(
                                    q2,
                                    csv[:, v, 1:SEGS + 1, D - 1],
                                    csv[:, v, 0:SEGS, D - 1])
                        t_idx += gsize
                        if t_idx % tb == 0:
                            flush_batch(b)

        if reps == 1:
            emit_main()
        else:
            with tc.For_i(0, reps, 1):
                emit_main()

        nc.scalar.activation(ln_buf[:], s_buf[:], AF.Ln,
                             accum_out=res[:, 0:1])
        nc.sync.dma_start(out, res[:])

    if not nc.is_finalized():
        nc.finalize()
    return nc


def host_params(means, cov_parts, log_weights):
    A = np.asarray(cov_parts, dtype=np.float64)
    mu = np.asarray(means, dtype=np.float64)
    w = np.asarray(log_weights, dtype=np.float64)

    cov = np.einsum('kij,klj->kil', A, A)
    L = np.linalg.cholesky(cov)
    eye = np.eye(D, dtype=np.float64)
    M = np.stack([np.linalg.solve(L[k], eye) for k in range(K)])
    b = np.einsum('kij,kj->ki', M, mu)
    logdet = np.log(np.diagonal(L, axis1=1, axis2=2)).sum(axis=1)
    c = -0.5 * D * LOG_2PI - logdet + w ** 2

    G = np.zeros((CPART, K * D), dtype=np.float64)
    for k in range(K):
        cols = slice(k * D, (k + 1) * D)
        G[0:D, cols] = M[k].T
        G[D, cols] = -b[k]
    G = np.ascontiguousarray(G.astype(np.float16))

    W = np.exp(c + SHIFT).astype(np.float32)
    wrep = np.ascontiguousarray(
        np.broadcast_to(W, (128, K))).astype(np.float32)

    lse_pad = np.log(np.sum(np.exp(c - 0.5 * (b ** 2).sum(axis=1))))
    logs0 = SHIFT + lse_pad
    return G, wrep, logs0


def build_t(data_core: np.ndarray, ng: int = NG) -> np.ndarray:
    npts = data_core.shape[0]
    npc = GROUPS * ng
    pad = npc - npts
    assert pad >= 0
    x = np.empty((npc, D), dtype=np.float16)
    x[:npts] = data_core.astype(np.float16)
    if pad:
        x[npts:] = 0.0
    xg = x.reshape(GROUPS, ng, D)
    t = np.empty((GROUPS, CPART, ng), dtype=np.float16)
    t[:, :D, :] = xg.transpose(0, 2, 1)
    t[:, D, :] = 1.0
    return np.ascontiguousarray(t.reshape(GROUPS * CPART, ng))


def _get_module():
    key = (TPG, WCHUNK, TB, PATTERN, FOLD_PATTERN)
    if key not in _MODULE_CACHE:
        _MODULE_CACHE[key] = build_module()
    return _MODULE_CACHE[key]


def build_in_maps(data, means, cov_parts, log_weights):
    data = np.asarray(data)
    assert data.shape == (N_TOTAL, D), data.shape
    G, wrep, logs0 = host_params(means, cov_parts, log_weights)
    in_maps = []
    for core in range(NCORES):
        shard = data[core * NC:(core + 1) * NC]
        in_maps.append({"t": build_t(shard), "g": G, "wrep": wrep})
    return in_maps


def run(data, means, cov_parts, log_weights, trace=False, **trace_kwargs):
    nc = _get_module()
    G, wrep, logs0 = host_params(means, cov_parts, log_weights)
    in_maps = build_in_maps(data, means, cov_parts, log_weights)
    res = run_bass_kernel_spmd(nc, in_maps, core_ids=list(range(NCORES)),
                               trace=trace, **trace_kwargs)

    total = 0.0
    for r in res.results:
        total += r["out"].astype(np.float64).sum()
    npad = NCORES * (NPC - NC)
    answer = (total - npad * logs0 - N_TOTAL * SHIFT) / N_TOTAL
    return np.float32(answer), res


def kernel(data, means, cov_parts, log_weights):
    ans, _ = run(data, means, cov_parts, log_weights, trace=False)
    return ans
